# revision 1
# baseline (speedup 1.0000x reference)
"""Trainium2 Bass kernel for nn_LogicGatedSpikingSelfAttention.

Sharding: channel/head-parallel over 8 cores. Each core owns 128 output
channels = 2 heads for the q/k/v branches (BN stats fully local, since
stats are per-channel over all tokens), runs attention for its 2 heads
over all 4 batches locally, and computes a 128-output-channel slice of
the projection. One AllGather moves the binary attention spikes (+ per-
head energies for the logic gate) between the attention and projection
stages; the gate is folded into the projection weights after the gather
(exact: gate is {0,1}).

Numerics: all big matmuls in bf16. The attention is exact in integers
(spikes are {0,1}: counts accumulate exactly in fp32 PSUM, and the
attn-LIF threshold reduces to an integer compare S >= 2^0.75). The LIF
forward pass is a pure Heaviside, so each branch reduces to
Y >= m + (2-beta)/gamma * sqrt(var+eps) with per-channel scalars.
"""
import numpy as np
import ml_dtypes

import concourse.bass as bass
import concourse.bacc as bacc
import concourse.tile as tile
from concourse import mybir
from concourse.bass_utils import run_bass_kernel_spmd

NCORES = 8
B, NSEQ, D, H = 4, 1024, 1024, 16
HD = D // H            # 64 head dim
CH = D // NCORES       # 128 channels per core
TOK = B * NSEQ         # 4096 tokens
KT = D // 128          # 8 contraction tiles
EPS = 1e-5
S_TH = float(2.0 ** 0.75)   # x_attn >= 1  <=>  S >= hd**0.125 = 2^0.75
SPIKE_N = 128 * TOK         # flat payload: spikes then 8 energy slots
PAYLEN = SPIKE_N + 8
F32 = mybir.dt.float32
BF16 = mybir.dt.bfloat16
BF = ml_dtypes.bfloat16

_CACHE = {}


def _build():
    nc = bacc.Bacc("TRN2", target_bir_lowering=False, debug=False,
                   num_devices=NCORES)
    inp = {}
    def din(name, shape, dt=BF16):
        inp[name] = nc.dram_tensor(name, shape, dt, kind="ExternalInput")
        return inp[name]

    din("xT",  [128, KT * TOK])          # host pre-tiled: [p, (t n)]
    din("wq",  [128, KT * CH]); din("wk", [128, KT * CH])
    din("wv",  [128, KT * CH]); din("wp", [128, KT * CH])
    for nm in ("tq", "tk", "tv", "tp", "bq", "bk", "bv", "bp"):
        din(nm, [CH, 1], F32)
    din("wgr", [H, H], F32)              # lhsT: [h, h'] = sum_r Wg[h', h+16r]/1024
    din("bgr", [H, 1], F32)
    din("i2e", [CH, 2], F32)             # [p, j] = (p//64==j)
    din("i16", [H, KT * 128], F32)       # [h, (t m)] = (t*128+m)//64 == h
    din("idn", [128, 128])               # identity for PE transpose
    outT = nc.dram_tensor("outT", [CH, TOK], BF16, kind="ExternalOutput")

    with tile.TileContext(nc) as tc:
        with tc.tile_pool(name="consts", bufs=1) as consts, \
             tc.tile_pool(name="spikes", bufs=1) as spk_pool, \
             tc.tile_pool(name="dram", bufs=1, space="DRAM") as dram:
            _body(tc, inp, outT, consts, spk_pool, dram)
    nc.compile()
    return nc


def _body(tc, inp, outT, consts, spk_pool, dram):
    nc = tc.nc
    V, SC, GP, TE = nc.vector, nc.scalar, nc.gpsimd, nc.tensor
    AF = mybir.ActivationFunctionType
    OP = mybir.AluOpType
    DENG = [nc.sync, nc.scalar, nc.gpsimd]

    # ---- constants / weights to SBUF (all host-contiguous) ----
    w_sb = {}
    for i, nm in enumerate(("wq", "wk", "wv", "wp")):
        t = consts.tile([128, KT, CH], BF16, name=f"{nm}_sb")
        DENG[i % 3].dma_start(
            t[:], inp[nm].ap().rearrange("p (t m) -> p t m", t=KT))
        w_sb[nm] = t
    small = {}
    for nm in ("tq", "tk", "tv", "tp", "bq", "bk", "bv", "bp", "bgr"):
        t = consts.tile([inp[nm].shape[0], 1], F32, name=f"{nm}_sb")
        nc.sync.dma_start(t[:], inp[nm].ap())
        small[nm] = t
    wgr_sb = consts.tile([H, H], F32)
    nc.sync.dma_start(wgr_sb[:], inp["wgr"].ap())
    i2e_sb = consts.tile([CH, 2], F32)
    nc.sync.dma_start(i2e_sb[:], inp["i2e"].ap())
    i16_sb = consts.tile([H, KT, 128], F32)
    nc.sync.dma_start(i16_sb[:],
                      inp["i16"].ap().rearrange("h (t m) -> h t m", t=KT))
    idn_sb = consts.tile([128, 128], BF16)
    nc.scalar.dma_start(idn_sb[:], inp["idn"].ap())
    eps_sb = consts.tile([128, 1], F32)
    V.memset(eps_sb[:], EPS)

    # ---- persistent spike tensors ----
    spA = {nm: spk_pool.tile([128, TOK], BF16, name=f"sp{nm}A")
           for nm in ("q", "k", "v")}
    sp2 = {nm: spk_pool.tile([HD, 2, TOK], BF16, name=f"sp2{nm}")
           for nm in ("q", "k")}
    vnat = spk_pool.tile([128, 32, 128], BF16)          # [tok, b*8+mt, ch]
    payload = spk_pool.tile([HD, 2, TOK], BF16)         # [d, h, tok] spikes

    # ================= branches (q, k, v) =================
    with tc.tile_pool(name="xts_p", bufs=1) as xts_p, \
         tc.tile_pool(name="ybig", bufs=2) as ybig, \
         tc.tile_pool(name="stps", bufs=2) as stp:
        xts = xts_p.tile([128, KT, TOK], BF16)
        nc.gpsimd.dma_start(
            xts[:], inp["xT"].ap().rearrange("p (t n) -> p t n", t=KT))

        for nm in ("q", "k", "v"):
            Y = ybig.tile([128, TOK], F32, tag="Y")
            # weight-stationary: kt outer, 8 PSUM banks accumulate
            with tc.tile_pool(name=f"brps_{nm}", bufs=1, space="PSUM") as brps:
                ps = [brps.tile([128, 512], F32, name=f"ps{nm}{i}")
                      for i in range(8)]
                for kt in range(KT):
                    for nck in range(8):
                        TE.matmul(ps[nck][:], w_sb["w" + nm][:, kt, :],
                                  xts[:, kt, nck * 512:(nck + 1) * 512],
                                  start=(kt == 0), stop=(kt == KT - 1))
                for nck in range(8):
                    if nck % 2:
                        V.tensor_scalar(Y[:, nck * 512:(nck + 1) * 512],
                                        ps[nck][:], small["b" + nm][:],
                                        None, OP.add)
                    else:
                        SC.activation(Y[:, nck * 512:(nck + 1) * 512],
                                      ps[nck][:], AF.Identity,
                                      bias=small["b" + nm][:])
            stats = stp.tile([128, 8, 6], F32, tag="stats")
            for i in range(8):
                V.bn_stats(stats[:, i, :], Y[:, i * 512:(i + 1) * 512])
            mv = stp.tile([128, 2], F32, tag="mv")
            V.bn_aggr(mv[:], stats[:])
            std = stp.tile([128, 1], F32, tag="std")
            SC.activation(std[:], mv[:, 1:2], AF.Sqrt, bias=eps_sb[:])
            thr = stp.tile([128, 1], F32, tag="thr")
            V.tensor_tensor(thr[:], std[:], small["t" + nm][:], OP.mult)
            V.tensor_tensor(thr[:], thr[:], mv[:, 0:1], OP.add)
            V.tensor_scalar(spA[nm][:], Y[:], thr[:], None, OP.is_ge)

        # head-split q, k for attention operand layout (base partition 0)
        for i, nm in enumerate(("q", "k")):
            for h in range(2):
                DENG[(i * 2 + h) % 3].dma_start(
                    sp2[nm][:, h, :], spA[nm][h * HD:(h + 1) * HD, :])

        # v -> natural (token, channel) layout via PE transpose
        with tc.tile_pool(name="tps", bufs=4, space="PSUM") as tps:
            for i in range(32):
                vt = tps.tile([128, 128], BF16, tag="vt")
                TE.transpose(vt[:], spA["v"][:, i * 128:(i + 1) * 128], idn_sb[:])
                if i % 2 == 0:
                    V.tensor_copy(vnat[:, i, :], vt[:])
                else:
                    SC.activation(vnat[:, i, :], vt[:], AF.Copy)

    # ================= energy =================
    e_sb = spk_pool.tile([2, B], BF16)
    with tc.tile_pool(name="enps", bufs=1, space="PSUM") as enps, \
         tc.tile_pool(name="entmp", bufs=1) as entmp:
        prod = entmp.tile([128, TOK], BF16)
        V.tensor_tensor(prod[:], spA["q"][:], spA["k"][:], OP.mult)
        ech = entmp.tile([128, B], F32)
        V.reduce_sum(ech[:], prod[:].rearrange("p (b n) -> p b n", b=B),
                     axis=mybir.AxisListType.X)
        e_ps = enps.tile([2, B], F32)
        TE.matmul(e_ps[:], i2e_sb[:], ech[:], start=True, stop=True)
        V.tensor_copy(e_sb[:], e_ps[:])

    # ================= attention =================
    with tc.tile_pool(name="cps", bufs=3, space="PSUM") as cps, \
         tc.tile_pool(name="sps", bufs=4, space="PSUM") as sps, \
         tc.tile_pool(name="csb", bufs=4) as csb:
        for b in range(B):
            for h in range(2):
                s_ps = [sps.tile([HD, 512], F32, tag="sps", name=f"s_ps{b}{h}{i}")
                        for i in range(2)]
                for mt in range(8):
                    m0 = b * NSEQ + mt * 128
                    for ncn in range(2):
                        n0 = b * NSEQ + ncn * 512
                        c_ps = cps.tile([128, 512], F32, tag="cps")
                        TE.matmul(c_ps[:], sp2["k"][:, h, m0:m0 + 128],
                                  sp2["q"][:, h, n0:n0 + 512],
                                  start=True, stop=True)
                        c_sb = csb.tile([128, 512], BF16, tag="csb")
                        if (mt * 2 + ncn) % 4 == 3:
                            SC.activation(c_sb[:], c_ps[:], AF.Copy)
                        else:
                            V.tensor_copy(c_sb[:], c_ps[:])
                        TE.matmul(s_ps[ncn][:],
                                  vnat[:, b * 8 + mt, h * HD:(h + 1) * HD],
                                  c_sb[:], start=(mt == 0), stop=(mt == 7))
                for ncn in range(2):
                    n0 = b * NSEQ + ncn * 512
                    V.tensor_scalar(payload[:, h, n0:n0 + 512], s_ps[ncn][:],
                                    S_TH, None, OP.is_ge)

    # ================= AllGather (flat, contiguous) =================
    pay_d = dram.tile([PAYLEN], BF16)
    gath_d = dram.tile([NCORES, PAYLEN], BF16, addr_space="Shared")
    for h in range(2):
        DENG[h].dma_start(
            pay_d[h * HD * TOK:(h + 1) * HD * TOK].rearrange(
                "(p n) -> p n", p=HD),
            payload[:, h, :])
    nc.sync.dma_start(
        pay_d[SPIKE_N:SPIKE_N + 8].rearrange("(p w) -> p w", p=2), e_sb[:])
    GP.collective_compute("AllGather", OP.bypass,
                          ins=[pay_d.opt()], outs=[gath_d.opt()],
                          replica_groups=[list(range(NCORES))])

    # ================= gate -> gated proj weights =================
    with tc.tile_pool(name="gtmp", bufs=1) as gtmp, \
         tc.tile_pool(name="post", bufs=1) as post, \
         tc.tile_pool(name="pstat", bufs=1) as pstat:
        with tc.tile_pool(name="gtps", bufs=2, space="PSUM") as gtps:
            eg_bf = gtmp.tile([H, B], BF16)
            nc.sync.dma_start(
                eg_bf[:],
                gath_d[:, SPIKE_N:SPIKE_N + 8].rearrange(
                    "c (p w) -> c p w", p=2))
            eg = gtmp.tile([H, B], F32)
            V.tensor_copy(eg[:], eg_bf[:])
            g_ps = gtps.tile([H, B], F32, tag="gps")
            TE.matmul(g_ps[:], wgr_sb[:], eg[:], start=True, stop=True)
            gate = gtmp.tile([H, B], F32)
            V.tensor_scalar(gate[:], g_ps[:], small["bgr"][:], 0.5,
                            OP.add, OP.is_ge)
            gv = gtmp.tile([128, KT, B], F32)
            for t in range(KT):
                gv_ps = gtps.tile([128, B], F32, tag="gvps")
                TE.matmul(gv_ps[:], i16_sb[:, t, :], gate[:],
                          start=True, stop=True)
                V.tensor_copy(gv[:, t, :], gv_ps[:])
            wpg = post.tile([128, KT, B, 128], BF16)
            for t in range(KT):
                for b in range(B):
                    if (t * B + b) % 2:
                        V.tensor_scalar(wpg[:, t, b, :], w_sb["wp"][:, t, :],
                                        gv[:, t, b:b + 1], None, OP.mult)
                    else:
                        SC.activation(wpg[:, t, b, :], w_sb["wp"][:, t, :],
                                      AF.Identity, scale=gv[:, t, b:b + 1])

        # ================= projection =================
        rhs = [post.tile([128, TOK], BF16, name=f"rhs{t}") for t in range(KT)]
        for t in range(KT):
            DENG[t % 3].dma_start(
                rhs[t][:],
                gath_d[t, 0:SPIKE_N].rearrange("(p n) -> p n", p=128))
        Yp = post.tile([128, TOK], F32)
        with tc.tile_pool(name="ppps", bufs=1, space="PSUM") as ppps:
            pp = [ppps.tile([128, 512], F32, name=f"pp{i}") for i in range(8)]
            for t in range(KT):
                for b in range(B):
                    for ncn in range(2):
                        n0 = b * NSEQ + ncn * 512
                        TE.matmul(pp[b * 2 + ncn][:], wpg[:, t, b, :],
                                  rhs[t][:, n0:n0 + 512],
                                  start=(t == 0), stop=(t == KT - 1))
            for i in range(8):
                if i % 2:
                    V.tensor_scalar(Yp[:, i * 512:(i + 1) * 512], pp[i][:],
                                    small["bp"][:], None, OP.add)
                else:
                    SC.activation(Yp[:, i * 512:(i + 1) * 512], pp[i][:],
                                  AF.Identity, bias=small["bp"][:])
        stats = pstat.tile([128, 8, 6], F32)
        for i in range(8):
            V.bn_stats(stats[:, i, :], Yp[:, i * 512:(i + 1) * 512])
        mv = pstat.tile([128, 2], F32)
        V.bn_aggr(mv[:], stats[:])
        std = pstat.tile([128, 1], F32)
        SC.activation(std[:], mv[:, 1:2], AF.Sqrt, bias=eps_sb[:])
        thr = pstat.tile([128, 1], F32)
        V.tensor_tensor(thr[:], std[:], small["tp"][:], OP.mult)
        V.tensor_tensor(thr[:], thr[:], mv[:, 0:1], OP.add)
        osb = pstat.tile([128, TOK], BF16)
        V.tensor_scalar(osb[:], Yp[:], thr[:], None, OP.is_ge)
        nc.sync.dma_start(outT.ap(), osb[:])


def _tile_rows(a):
    # (8*128, N) -> (128, 8*N) so the SBUF [p, (t n)] load is contiguous
    n = a.shape[1]
    return np.ascontiguousarray(
        a.reshape(KT, 128, n).transpose(1, 0, 2).reshape(128, KT * n))


def _prep_inputs(inputs):
    x = np.asarray(inputs["x"], np.float32)
    xT = _tile_rows(x.reshape(TOK, D).T.astype(BF))
    Wg = np.asarray(inputs["Wg"], np.float64)
    wgr = (Wg.reshape(H, HD, H).sum(axis=1).T / 1024.0).astype(np.float32)
    wgr = np.ascontiguousarray(wgr)                     # [h, h']
    bgr = np.asarray(inputs["bg"], np.float32).reshape(H, 1)
    i2e = np.zeros((CH, 2), np.float32)
    i2e[0:HD, 0] = 1.0
    i2e[HD:CH, 1] = 1.0
    i16 = np.zeros((H, D), np.float32)
    for h in range(H):
        i16[h, h * HD:(h + 1) * HD] = 1.0
    i16 = np.ascontiguousarray(
        i16.reshape(H, KT, 128).reshape(H, KT * 128))
    idn = np.eye(128, dtype=BF)
    in_maps = []
    for c in range(NCORES):
        sl = slice(CH * c, CH * c + CH)
        m = {"xT": xT, "wgr": wgr, "bgr": bgr, "i2e": i2e, "i16": i16,
             "idn": idn}
        for nm in ("q", "k", "v", "p"):
            W = np.asarray(inputs[f"W{nm}"], np.float32)
            m["w" + nm] = _tile_rows(W[sl, :].T.astype(BF))
            g = np.asarray(inputs[f"g{nm}"], np.float32)[sl]
            be = np.asarray(inputs[f"beta{nm}"], np.float32)[sl]
            m["t" + nm] = ((2.0 - be) / g).reshape(CH, 1).astype(np.float32)
            m["b" + nm] = np.asarray(
                inputs[f"b{nm}"], np.float32)[sl].reshape(CH, 1)
        in_maps.append(m)
    return in_maps


def _run(inputs, trace=False):
    if "nc" not in _CACHE:
        _CACHE["nc"] = _build()
    nc = _CACHE["nc"]
    in_maps = _prep_inputs(inputs)
    res = run_bass_kernel_spmd(nc, in_maps, core_ids=list(range(NCORES)),
                               trace=trace)
    out = np.empty((TOK, D), np.float32)
    for c in range(NCORES):
        out[:, CH * c:CH * c + CH] = res.results[c]["outT"].astype(np.float32).T
    return out.reshape(B, NSEQ, D), res


def kernel(**inputs) -> np.ndarray:
    out, _ = _run(inputs, trace=False)
    return out



# revision 18
# speedup vs baseline: 1.4351x; 1.4351x over previous
"""Trainium2 Bass kernel for nn_LogicGatedSpikingSelfAttention (v2).

Structure (8 cores):
- q/k/v branches channel-sharded: core c owns output channels
  [128c, 128c+128) = heads 2c, 2c+1; BN stats fully local.
- Attention uses linearity (no softmax): S = q @ (k^T v) per (batch,
  head) - O(N*hd^2) instead of O(N^2*hd), integer-exact on {0,1}
  spikes (kv counts <= ~7 here, bf16-exact).
- Projection token-sharded: an AllToAll moves each core's [128ch x
  512tok] spike slices (~0.5MB) instead of an 8.4MB AllGather. Gate
  energies ride the payload as an exact fp8 hi+lo split; the gate is
  applied receiver-side to the proj rhs. Proj BN stats via an 8KB
  AllReduce of per-core (sum, sumsq).
- Branch + proj GEMMs run in fp8 e4m3 with DoubleRow perf mode
  (256-wide contraction per matmul). Weights pre-scaled by 8 on the
  host (spike threshold compare is scale-invariant; the linear bias
  cancels inside BatchNorm). Attention matmuls/transposes in bf16.
"""
import numpy as np
import ml_dtypes

import concourse.bass as bass
import concourse.bacc as bacc
import concourse.tile as tile
from concourse import mybir
from concourse.bass_utils import run_bass_kernel_spmd

NCORES = 8
B, NSEQ, D, H = 4, 1024, 1024, 16
HD = D // H            # 64 head dim
CH = D // NCORES       # 128 channels per core
TOK = B * NSEQ         # 4096 tokens
KT = D // 128          # 8 contraction tiles of 128
KP = KT // 2           # 4 DoubleRow contraction pairs of 256
NC = TOK // 512        # 8 token chunks of 512
EPS = 1e-5
S_TH = float(2.0 ** 0.75)   # x_attn spike <=> S >= hd**0.125 = 2^0.75
PCOL = 520                  # payload row: 512 spikes + 8 energy cols
F32 = mybir.dt.float32
BF16 = mybir.dt.bfloat16
FP8 = mybir.dt.float8e4
BF = ml_dtypes.bfloat16
F8 = mybir.dt.np(mybir.dt.float8e4)
DR = mybir.MatmulPerfMode.DoubleRow

_CACHE = {}


def _build():
    nc = bacc.Bacc("TRN2", target_bir_lowering=False, debug=False,
                   num_devices=NCORES)
    inp = {}
    def din(name, shape, dt):
        inp[name] = nc.dram_tensor(name, shape, dt, kind="ExternalInput")

    din("xq", [128, NC * KT * 512], FP8)     # [p, (nc kt n)] = xT pre-tiled
    din("wq", [128, KT * CH], FP8)           # [p, (kt j)] = 8*W[c0+j, kt*128+p]
    din("wk", [128, KT * CH], FP8)
    din("wv", [128, KT * CH], FP8)
    din("wp", [128, KT * D], FP8)            # full Wp^T: [p,(ct j)] = 8*Wp[j, ct*128+p]
    for nm in ("tq", "tk", "tv"):
        din(nm, [CH, 1], F32)                # (2-beta)/gamma, core's channel slice
    din("tp", [CH, KT], F32)                 # proj (2-beta)/gamma, all 1024 ch
    din("wgr", [NC, 2 * H], F32)             # [s, j*16+h'] = wgr0[2s+j, h']
    din("bgr", [H, 1], F32)
    din("i2e", [CH, 2], F32)                 # [p, j] = (p//64 == j)
    din("selp", [H, 128], F32)               # [h, p] = (p//64 == h%2)
    din("i8t", [H, KT], F32)                 # [h, t] = (h//2 == t)
    din("bsel", [NC, 2 * B], F32)            # per-core one-hot of own batch x2
    din("mbd", [128, 128], F32)              # block-diag 64x64 ones mask
    din("idn", [128, 128], BF16)             # identity for PE transpose
    outT = nc.dram_tensor("outT", [128, KT * 512], BF16, kind="ExternalOutput")

    with tile.TileContext(nc) as tc:
        with tc.tile_pool(name="consts", bufs=1) as consts, \
             tc.tile_pool(name="spk", bufs=1) as spk_pool, \
             tc.tile_pool(name="dram", bufs=1, space="DRAM") as dram:
            _body(tc, inp, outT, consts, spk_pool, dram)
    nc.compile()
    return nc


def _body(tc, inp, outT, consts, spk_pool, dram):
    nc = tc.nc
    V, SC, GP, TE, SY = nc.vector, nc.scalar, nc.gpsimd, nc.tensor, nc.sync
    AF = mybir.ActivationFunctionType
    OP = mybir.AluOpType
    RG = [list(range(NCORES))]

    # ---- constants / weights to SBUF ----
    w_sb = {}
    for i, nm in enumerate(("wq", "wk", "wv")):
        t = consts.tile([128, KP, 2, CH], FP8, name=f"{nm}_sb")
        SC.dma_start(t[:], inp[nm].ap().rearrange(
            "p (a r m) -> p a r m", a=KP, r=2))
        w_sb[nm] = t
    wp_sb = consts.tile([128, KP, 2, D], FP8)
    small = {}
    for nm in ("tq", "tk", "tv", "tp", "wgr", "bgr", "i2e", "selp", "i8t",
               "bsel", "mbd"):
        t = consts.tile(list(inp[nm].shape), F32, name=f"{nm}_sb")
        GP.dma_start(t[:], inp[nm].ap())
        small[nm] = t
    idn_sb = consts.tile([128, 128], BF16)
    GP.dma_start(idn_sb[:], inp["idn"].ap())
    eps_sb = consts.tile([128, 1], F32)
    V.memset(eps_sb[:], EPS)

    # ---- persistent spike / payload tensors ----
    spq = spk_pool.tile([128, TOK], BF16, name="spq")
    spk = spk_pool.tile([128, TOK], BF16, name="spk")
    spv = spk_pool.tile([128, TOK], BF16, name="spv")
    pay_sb = spk_pool.tile([128, NC, PCOL], FP8, name="pay_sb")
    V.memset(pay_sb[:, :, 512:PCOL], 0.0)

    # ================= branches (k, v, q) =================
    brs = ("k", "v", "q")
    with tc.tile_pool(name="xqp", bufs=1) as xqp, \
         tc.tile_pool(name="ybig", bufs=1) as ybig, \
         tc.tile_pool(name="stp", bufs=1) as stp:
        xq = xqp.tile([128, NC, KP, 2, 512], FP8)
        xq_ap = inp["xq"].ap().rearrange(
            "p (c a r n) -> p c a r n", c=NC, a=KP, r=2)
        for c in range(NC):
            (SY, SC)[c % 2].dma_start(xq[:, c], xq_ap[:, c])
        SC.dma_start(wp_sb[:], inp["wp"].ap().rearrange(
            "p (a r m) -> p a r m", a=KP, r=2))

        Y = {nm: ybig.tile([128, TOK], F32, name=f"Y{nm}") for nm in brs}
        st = {nm: stp.tile([128, NC, 6], F32, name=f"st{nm}") for nm in brs}
        with tc.tile_pool(name="brps", bufs=1, space="PSUM") as brps:
            ps = [brps.tile([128, 512], F32, name=f"ps{i}") for i in range(6)]
            for c in range(NC):
                g = c % 2
                for bi, nm in enumerate(brs):
                    p = ps[g * 3 + bi]
                    for a in range(KP):
                        TE.matmul(p[:], w_sb["w" + nm][:, a], xq[:, c, a],
                                  start=(a == 0), stop=(a == KP - 1),
                                  perf_mode=DR)
                    SC.activation(Y[nm][:, c * 512:(c + 1) * 512], p[:],
                                  AF.Copy)
                    V.bn_stats(st[nm][:, c], p[:])

        # stats -> per-channel thresholds -> spikes
        sp = {"k": spk, "v": spv, "q": spq}
        for nm in brs:
            mv = stp.tile([128, 2], F32, name=f"mv{nm}")
            V.bn_aggr(mv[:], st[nm][:])
            sd = stp.tile([128, 1], F32, name=f"sd{nm}")
            SC.activation(sd[:], mv[:, 1:2], AF.Sqrt, bias=eps_sb[:])
            th = stp.tile([128, 1], F32, name=f"th{nm}")
            V.tensor_tensor(th[:], sd[:], small["t" + nm][:], OP.mult)
            V.tensor_tensor(th[:], th[:], mv[:, 0:1], OP.add)
            eng = V if nm == "k" else GP
            eng.tensor_scalar(sp[nm][:], Y[nm][:], th[:], None, OP.is_ge)

    # ================= energy (own 2 heads, all batches) =================
    ea8 = spk_pool.tile([2, B], FP8, name="ea8")
    eb8 = spk_pool.tile([2, B], FP8, name="eb8")
    with tc.tile_pool(name="entmp", bufs=1) as entmp, \
         tc.tile_pool(name="enps", bufs=1, space="PSUM") as enps:
        prod = entmp.tile([128, TOK], BF16)
        GP.tensor_tensor(prod[:], spq[:], spk[:], OP.mult)
        ech = entmp.tile([128, B], F32)
        V.reduce_sum(ech[:], prod[:].rearrange("p (b n) -> p b n", b=B),
                     axis=mybir.AxisListType.X)
        e_ps = enps.tile([2, B], F32)
        TE.matmul(e_ps[:], small["i2e"][:], ech[:], start=True, stop=True)
        e_sb = entmp.tile([2, B], F32)
        V.tensor_copy(e_sb[:], e_ps[:])
        # exact fp8 split: e = ea + eb, ea = fp8(e), eb = e - ea (small int)
        V.tensor_copy(ea8[:], e_sb[:])
        ea32 = entmp.tile([2, B], F32)
        V.tensor_copy(ea32[:], ea8[:])
        eb = entmp.tile([2, B], F32)
        V.tensor_tensor(eb[:], e_sb[:], ea32[:], OP.subtract)
        V.tensor_copy(eb8[:], eb[:])
    for c in range(NC):
        V.tensor_copy(pay_sb[0:2, c, 512:516], ea8[:])
        V.tensor_copy(pay_sb[0:2, c, 516:520], eb8[:])

    # ================= attention: S = q (k^T v), spikes ==================
    with tc.tile_pool(name="nat", bufs=1) as nat, \
         tc.tile_pool(name="tps", bufs=2, space="PSUM") as tps, \
         tc.tile_pool(name="kvps", bufs=2, space="PSUM") as kvps, \
         tc.tile_pool(name="sps", bufs=2, space="PSUM") as sps, \
         tc.tile_pool(name="kvsb", bufs=1) as kvsb:
        knat = nat.tile([128, 32, 128], BF16)
        vnat = nat.tile([128, 32, 128], BF16)
        kvm = [kvsb.tile([128, 128], BF16, name=f"kvm{b}") for b in range(B)]

        def transposes(b):
            for half in range(2):
                tpk = tps.tile([128, 512], BF16, tag="tpk")
                tpv = tps.tile([128, 512], BF16, tag="tpv")
                for mi in range(4):
                    gt = b * 8 + half * 4 + mi
                    sl = slice(mi * 128, (mi + 1) * 128)
                    # one accumulation group per PSUM bank (disjoint
                    # columns): zero-on-first-write applies to the whole
                    # bank, then the remaining transposes land in it
                    TE.matmul(tpk[:, sl], spk[:, gt * 128:(gt + 1) * 128],
                              idn_sb[:], is_transpose=True,
                              start=(mi == 0), stop=(mi == 3),
                              skip_group_check=True)
                    TE.matmul(tpv[:, sl], spv[:, gt * 128:(gt + 1) * 128],
                              idn_sb[:], is_transpose=True,
                              start=(mi == 0), stop=(mi == 3),
                              skip_group_check=True)
                g0 = b * 8 + half * 4
                SC.activation(knat[:, g0:g0 + 4], tpk[:].rearrange(
                    "p (t m) -> p t m", t=4), AF.Copy)
                V.tensor_copy(vnat[:, g0:g0 + 4], tpv[:].rearrange(
                    "p (t m) -> p t m", t=4))

        def kv_stage(b):
            kv_ps = kvps.tile([128, 128], F32, tag="kv")
            for mt in range(8):
                TE.matmul(kv_ps[:], knat[:, b * 8 + mt], vnat[:, b * 8 + mt],
                          start=(mt == 0), stop=(mt == 7))
            V.tensor_tensor(kvm[b][:], kv_ps[:], small["mbd"][:], OP.mult)

        def s_stage(b):
            for nn in range(2):
                cdst = b * 2 + nn
                s_ps = sps.tile([128, 512], F32, tag="sps")
                TE.matmul(s_ps[:], kvm[b][:],
                          spq[:, cdst * 512:(cdst + 1) * 512],
                          start=True, stop=True)
                V.tensor_scalar(pay_sb[:, cdst, 0:512], s_ps[:], S_TH,
                                None, OP.is_ge)

        # software-pipelined over batches to keep PE busy
        transposes(0)
        transposes(1)
        kv_stage(0)
        transposes(2)
        kv_stage(1)
        s_stage(0)
        transposes(3)
        kv_stage(2)
        s_stage(1)
        kv_stage(3)
        s_stage(2)
        s_stage(3)

    # ================= AllToAll =================
    pay_d = dram.tile([NC, 128, PCOL], FP8)
    recv_d = dram.tile([NC, 128, PCOL], FP8)
    GP.dma_start(pay_d[:].rearrange("c p w -> p c w"), pay_sb[:])
    GP.collective_compute("AllToAll", OP.bypass,
                          ins=[pay_d.opt()], outs=[recv_d.opt()],
                          replica_groups=RG)

    # ================= gate + projection (own 512 tokens) ================
    with tc.tile_pool(name="phC", bufs=1) as phC:
        rsb = phC.tile([128, KP, 2, 512], FP8)
        rc_ap = recv_d[:].rearrange("(a r) p w -> p a r w", a=KP)
        SY.dma_start(rsb[:], rc_ap[:, :, :, 0:512])
        gx = phC.tile([128, KT], F32)
        with tc.tile_pool(name="gps", bufs=1, space="PSUM") as gps:
            # gather energies [8 s, 2 j, 8]: cols 0:4 = ea[b], 4:8 = eb[b];
            # head h = 2s + j
            er8 = phC.tile([NC, 2, 2 * B], FP8, name="er8")
            SC.dma_start(er8[:], recv_d[:, 0:2, 512:520])
            er = phC.tile([NC, 2, 2 * B], F32, name="er")
            V.tensor_copy(er[:], er8[:])
            e2 = phC.tile([NC, 2, B], F32, name="e2")
            V.tensor_tensor(e2[:], er[:, :, 0:B], er[:, :, B:2 * B], OP.add)
            V.tensor_tensor(e2[:], e2[:], small["bsel"][:].rearrange(
                "s (j b) -> s j b", j=2), OP.mult)
            emy = phC.tile([NC, 2], F32, name="emy")
            V.reduce_sum(emy[:], e2[:], axis=mybir.AxisListType.X)
            g_ps = gps.tile([H, 1], F32, name="g_ps")
            wgr_ap = small["wgr"][:].rearrange("s (j h) -> s j h", j=2)
            TE.matmul(g_ps[:], wgr_ap[:, 0], emy[:, 0:1],
                      start=True, stop=False)
            TE.matmul(g_ps[:], wgr_ap[:, 1], emy[:, 1:2],
                      start=False, stop=True)
            gate = phC.tile([H, 1], F32)
            V.tensor_scalar(gate[:], g_ps[:], small["bgr"][:], 0.5,
                            OP.add, OP.is_ge)
            rhs_t = phC.tile([H, KT], F32)
            V.tensor_scalar(rhs_t[:], small["i8t"][:], gate[:], None, OP.mult)
            gx_ps = gps.tile([128, KT], F32, name="gx_ps")
            TE.matmul(gx_ps[:], small["selp"][:], rhs_t[:],
                      start=True, stop=True)
            V.tensor_copy(gx[:], gx_ps[:])
        for i in range(KT):
            V.tensor_scalar(rsb[:, i // 2, i % 2], rsb[:, i // 2, i % 2],
                            gx[:, i:i + 1], None, OP.mult)

        osb = phC.tile([128, KT, 512], BF16)
        with tc.tile_pool(name="ppps", bufs=1, space="PSUM") as ppps:
            pp = [ppps.tile([128, 512], F32, name=f"pp{o}") for o in range(KT)]
            for o in range(KT):
                for a in range(KP):
                    TE.matmul(pp[o][:], wp_sb[:, a, :, o * 128:(o + 1) * 128],
                              rsb[:, a], start=(a == 0), stop=(a == KP - 1),
                              perf_mode=DR)
            # per-channel partial stats over own 512 tokens
            pst = phC.tile([128, KT, 6], F32)
            mvp = phC.tile([128, KT, 2], F32)
            for o in range(KT):
                V.bn_stats(pst[:, o], pp[o][:])
                V.bn_aggr(mvp[:, o], pst[:, o])
            arf = phC.tile([128, 2, KT], F32)
            V.tensor_scalar(arf[:, 0], mvp[:, :, 0], 512.0, None, OP.mult)
            V.tensor_tensor(arf[:, 1], mvp[:, :, 0], mvp[:, :, 0], OP.mult)
            V.tensor_tensor(arf[:, 1], arf[:, 1], mvp[:, :, 1], OP.add)
            V.tensor_scalar(arf[:, 1], arf[:, 1], 512.0, None, OP.mult)
            ps_d = dram.tile([128, 2 * KT], F32)
            pr_d = dram.tile([128, 2 * KT], F32)
            SY.dma_start(ps_d[:].rearrange("p (s t) -> p s t", s=2), arf[:])
            GP.collective_compute("AllReduce", OP.add,
                                  ins=[ps_d.opt()], outs=[pr_d.opt()],
                                  replica_groups=RG)
            arb = phC.tile([128, 2, KT], F32)
            SY.dma_start(arb[:], pr_d[:].rearrange("p (s t) -> p s t", s=2))
            mean = phC.tile([128, KT], F32)
            V.tensor_scalar(mean[:], arb[:, 0], 1.0 / TOK, None, OP.mult)
            var = phC.tile([128, KT], F32)
            V.tensor_tensor(var[:], mean[:], mean[:], OP.mult)
            ssm = phC.tile([128, KT], F32)
            V.tensor_scalar(ssm[:], arb[:, 1], 1.0 / TOK, None, OP.mult)
            V.tensor_tensor(var[:], ssm[:], var[:], OP.subtract)
            sdp = phC.tile([128, KT], F32)
            SC.activation(sdp[:], var[:], AF.Sqrt, bias=eps_sb[:])
            thrp = phC.tile([128, KT], F32)
            V.tensor_tensor(thrp[:], sdp[:], small["tp"][:], OP.mult)
            V.tensor_tensor(thrp[:], thrp[:], mean[:], OP.add)
            for o in range(KT):
                V.tensor_scalar(osb[:, o], pp[o][:], thrp[:, o:o + 1],
                                None, OP.is_ge)
        SY.dma_start(outT.ap().rearrange("p (t n) -> p t n", t=KT), osb[:])


def _prep_inputs(inputs):
    x = np.asarray(inputs["x"], np.float32)
    # xq[p, nc, kt, n] = x^T[kt*128+p, nc*512+n]
    xt = np.ascontiguousarray(x.reshape(TOK, D).T)
    xq = np.ascontiguousarray(
        xt.reshape(KT, 128, NC, 512).transpose(1, 2, 0, 3)
    ).reshape(128, NC * KT * 512).astype(F8)

    def wtile(W):
        # [p, kt*ncols + j] = W[j, kt*128+p]  (W already scaled/sliced)
        ncols = W.shape[0]
        return np.ascontiguousarray(
            W.T.reshape(KT, 128, ncols).transpose(1, 0, 2)
        ).reshape(128, KT * ncols)

    Wp = np.asarray(inputs["Wp"], np.float32)
    wp8 = wtile(8.0 * Wp).astype(F8)
    gp = np.asarray(inputs["gp"], np.float32)
    bp = np.asarray(inputs["betap"], np.float32)
    tp = np.ascontiguousarray(
        ((2.0 - bp) / gp).reshape(KT, 128).T).astype(np.float32)

    Wg = np.asarray(inputs["Wg"], np.float64)
    wgr0 = (Wg.reshape(H, HD, H).sum(axis=1).T / float(NSEQ)).astype(
        np.float32)                              # [h, h']
    # wgr[s, j*16+h'] = wgr0[2s+j, h']
    wgr = np.ascontiguousarray(
        wgr0.reshape(NC, 2, H).transpose(0, 1, 2).reshape(NC, 2 * H))
    bgr = np.asarray(inputs["bg"], np.float32).reshape(H, 1)

    i2e = np.zeros((CH, 2), np.float32)
    i2e[0:HD, 0] = 1.0
    i2e[HD:CH, 1] = 1.0
    selp = np.zeros((H, 128), np.float32)
    for h in range(H):
        selp[h, (h % 2) * HD:(h % 2 + 1) * HD] = 1.0
    i8t = np.zeros((H, KT), np.float32)
    for h in range(H):
        i8t[h, h // 2] = 1.0
    mbd = np.zeros((128, 128), np.float32)
    mbd[0:HD, 0:HD] = 1.0
    mbd[HD:128, HD:128] = 1.0
    idn = np.eye(128, dtype=BF)

    in_maps = []
    for c in range(NCORES):
        sl = slice(CH * c, CH * c + CH)
        bsel = np.zeros((NC, 2, B), np.float32)
        bsel[:, :, c // 2] = 1.0
        bsel = bsel.reshape(NC, 2 * B)
        m = {"xq": xq, "wp": wp8, "tp": tp, "wgr": wgr, "bgr": bgr,
             "i2e": i2e, "selp": selp, "i8t": i8t, "bsel": bsel,
             "mbd": mbd, "idn": idn}
        for nm in ("q", "k", "v"):
            W = np.asarray(inputs[f"W{nm}"], np.float32)
            m["w" + nm] = wtile(8.0 * W[sl, :]).astype(F8)
            g = np.asarray(inputs[f"g{nm}"], np.float32)[sl]
            be = np.asarray(inputs[f"beta{nm}"], np.float32)[sl]
            m["t" + nm] = ((2.0 - be) / g).reshape(CH, 1).astype(np.float32)
        in_maps.append(m)
    return in_maps


def _run(inputs, trace=False):
    if "nc" not in _CACHE:
        _CACHE["nc"] = _build()
    nc = _CACHE["nc"]
    in_maps = _prep_inputs(inputs)
    res = run_bass_kernel_spmd(nc, in_maps, core_ids=list(range(NCORES)),
                               trace=trace)
    out = np.empty((TOK, D), np.float32)
    for c in range(NCORES):
        blk = res.results[c]["outT"].reshape(128, KT, 512).astype(np.float32)
        out[c * 512:(c + 1) * 512, :] = blk.transpose(2, 1, 0).reshape(512, D)
    return out.reshape(B, NSEQ, D), res


def kernel(**inputs) -> np.ndarray:
    out, _ = _run(inputs, trace=False)
    return out


# revision 21
# speedup vs baseline: 2.4810x; 1.7287x over previous
"""Trainium2 Bass kernel for nn_LogicGatedSpikingSelfAttention (v2).

Structure (8 cores):
- q/k/v branches channel-sharded: core c owns output channels
  [128c, 128c+128) = heads 2c, 2c+1; BN stats fully local.
- Attention uses linearity (no softmax): S = q @ (k^T v) per (batch,
  head) - O(N*hd^2) instead of O(N^2*hd), integer-exact on {0,1}
  spikes (kv counts <= ~7 here, bf16-exact).
- Projection token-sharded: an AllToAll moves each core's [128ch x
  512tok] spike slices (~0.5MB) instead of an 8.4MB AllGather. Gate
  energies ride the payload as an exact fp8 hi+lo split; the gate is
  applied receiver-side to the proj rhs. Proj BN stats via an 8KB
  AllReduce of per-core (sum, sumsq).
- Branch + proj GEMMs run in fp8 e4m3 with DoubleRow perf mode
  (256-wide contraction per matmul). Weights pre-scaled by 8 on the
  host (spike threshold compare is scale-invariant; the linear bias
  cancels inside BatchNorm). Attention matmuls/transposes in bf16.
"""
import numpy as np
import ml_dtypes

import concourse.bass as bass
import concourse.bacc as bacc
import concourse.tile as tile
from concourse import mybir
from concourse.bass_utils import run_bass_kernel_spmd

NCORES = 8
B, NSEQ, D, H = 4, 1024, 1024, 16
HD = D // H            # 64 head dim
CH = D // NCORES       # 128 channels per core
TOK = B * NSEQ         # 4096 tokens
KT = D // 128          # 8 contraction tiles of 128
KP = KT // 2           # 4 DoubleRow contraction pairs of 256
NC = TOK // 512        # 8 token chunks of 512
EPS = 1e-5
S_TH = float(2.0 ** 0.75)   # x_attn spike <=> S >= hd**0.125 = 2^0.75
PCOL = 520                  # payload row: 512 spikes + 8 energy cols
F32 = mybir.dt.float32
BF16 = mybir.dt.bfloat16
FP8 = mybir.dt.float8e4
BF = ml_dtypes.bfloat16
F8 = mybir.dt.np(mybir.dt.float8e4)
DR = mybir.MatmulPerfMode.DoubleRow

_CACHE = {}


def _build():
    nc = bacc.Bacc("TRN2", target_bir_lowering=False, debug=False,
                   num_devices=NCORES)
    inp = {}
    def din(name, shape, dt):
        inp[name] = nc.dram_tensor(name, shape, dt, kind="ExternalInput")

    din("xq", [128, NC * KT * 512], FP8)     # [p, (nc kt n)] = xT pre-tiled
    din("wq", [128, KT * CH], FP8)           # [p, (kt j)] = 8*W[c0+j, kt*128+p]
    din("wk", [128, KT * CH], FP8)
    din("wv", [128, KT * CH], FP8)
    din("wp", [128, KT * D], FP8)            # full Wp^T: [p,(ct j)] = 8*Wp[j, ct*128+p]
    for nm in ("tq", "tk", "tv"):
        din(nm, [CH, 1], F32)                # (2-beta)/gamma, core's channel slice
    din("tp", [CH, KT], F32)                 # proj (2-beta)/gamma, all 1024 ch
    din("wgr", [NC, 2 * H], F32)             # [s, j*16+h'] = wgr0[2s+j, h']
    din("bgr", [H, 1], F32)
    din("i2e", [CH, 2], F32)                 # [p, j] = (p//64 == j)
    din("selp", [H, 128], F32)               # [h, p] = (p//64 == h%2)
    din("i8t", [H, KT], F32)                 # [h, t] = (h//2 == t)
    din("bsel", [NC, 2 * B], F32)            # per-core one-hot of own batch x2
    din("mbd", [128, 128], F32)              # block-diag 64x64 ones mask
    din("idn", [128, 128], BF16)             # identity for PE transpose
    outT = nc.dram_tensor("outT", [128, KT * 512], BF16, kind="ExternalOutput")

    with tile.TileContext(nc) as tc:
        with tc.tile_pool(name="consts", bufs=1) as consts, \
             tc.tile_pool(name="spk", bufs=1) as spk_pool, \
             tc.tile_pool(name="dram", bufs=1, space="DRAM") as dram:
            _body(tc, inp, outT, consts, spk_pool, dram)
    nc.compile()
    return nc


def _body(tc, inp, outT, consts, spk_pool, dram):
    nc = tc.nc
    V, SC, GP, TE, SY = nc.vector, nc.scalar, nc.gpsimd, nc.tensor, nc.sync
    AF = mybir.ActivationFunctionType
    OP = mybir.AluOpType
    RG = [list(range(NCORES))]

    # ---- constants / weights to SBUF ----
    w_sb = {}
    for i, nm in enumerate(("wq", "wk", "wv")):
        t = consts.tile([128, KP, 2, CH], FP8, name=f"{nm}_sb")
        SC.dma_start(t[:], inp[nm].ap().rearrange(
            "p (a r m) -> p a r m", a=KP, r=2))
        w_sb[nm] = t
    wp_sb = consts.tile([128, KP, 2, D], FP8)
    small = {}
    for nm in ("tq", "tk", "tv", "tp", "wgr", "bgr", "i2e", "selp", "i8t",
               "bsel", "mbd"):
        t = consts.tile(list(inp[nm].shape), F32, name=f"{nm}_sb")
        GP.dma_start(t[:], inp[nm].ap())
        small[nm] = t
    idn_sb = consts.tile([128, 128], BF16)
    GP.dma_start(idn_sb[:], inp["idn"].ap())
    eps_sb = consts.tile([128, 1], F32)
    V.memset(eps_sb[:], EPS)

    # ---- persistent spike / payload tensors ----
    spq = spk_pool.tile([128, TOK], BF16, name="spq")
    spk = spk_pool.tile([128, TOK], BF16, name="spk")
    spv = spk_pool.tile([128, TOK], BF16, name="spv")
    pay_sb = spk_pool.tile([128, NC, PCOL], FP8, name="pay_sb")
    V.memset(pay_sb[:, :, 512:PCOL], 0.0)

    # ================= branches (k, v, q) =================
    brs = ("k", "v", "q")
    with tc.tile_pool(name="xqp", bufs=1) as xqp, \
         tc.tile_pool(name="ybig", bufs=1) as ybig, \
         tc.tile_pool(name="stp", bufs=1) as stp:
        xq = xqp.tile([128, NC, KP, 2, 512], FP8)
        xq_ap = inp["xq"].ap().rearrange(
            "p (c a r n) -> p c a r n", c=NC, a=KP, r=2)
        for c in range(NC):
            (SY, SC)[c % 2].dma_start(xq[:, c], xq_ap[:, c])
        SC.dma_start(wp_sb[:], inp["wp"].ap().rearrange(
            "p (a r m) -> p a r m", a=KP, r=2))

        Y = {nm: ybig.tile([128, TOK], BF16, name=f"Y{nm}") for nm in brs}
        st = {nm: stp.tile([128, NC, 6], F32, name=f"st{nm}") for nm in brs}
        with tc.tile_pool(name="brps", bufs=1, space="PSUM") as brps:
            ps = [brps.tile([128, 512], F32, name=f"ps{i}") for i in range(6)]
            for c in range(NC):
                g = c % 2
                for bi, nm in enumerate(brs):
                    p = ps[g * 3 + bi]
                    for a in range(KP):
                        TE.matmul(p[:], w_sb["w" + nm][:, a], xq[:, c, a],
                                  start=(a == 0), stop=(a == KP - 1),
                                  perf_mode=DR)
                    SC.activation(Y[nm][:, c * 512:(c + 1) * 512], p[:],
                                  AF.Copy)
                    V.bn_stats(st[nm][:, c], p[:])

        # stats -> per-channel thresholds -> spikes
        sp = {"k": spk, "v": spv, "q": spq}
        for nm in brs:
            mv = stp.tile([128, 2], F32, name=f"mv{nm}")
            V.bn_aggr(mv[:], st[nm][:])
            sd = stp.tile([128, 1], F32, name=f"sd{nm}")
            SC.activation(sd[:], mv[:, 1:2], AF.Sqrt, bias=eps_sb[:])
            th = stp.tile([128, 1], F32, name=f"th{nm}")
            V.tensor_tensor(th[:], sd[:], small["t" + nm][:], OP.mult)
            V.tensor_tensor(th[:], th[:], mv[:, 0:1], OP.add)
            V.tensor_scalar(sp[nm][:], Y[nm][:], th[:], None, OP.is_ge)

    # ================= energy (own 2 heads, all batches) =================
    ea8 = spk_pool.tile([2, B], FP8, name="ea8")
    eb8 = spk_pool.tile([2, B], FP8, name="eb8")
    with tc.tile_pool(name="entmp", bufs=1) as entmp, \
         tc.tile_pool(name="enps", bufs=1, space="PSUM") as enps:
        prod = entmp.tile([128, TOK], BF16)
        V.tensor_tensor(prod[:], spq[:], spk[:], OP.mult)
        ech = entmp.tile([128, B], F32)
        V.reduce_sum(ech[:], prod[:].rearrange("p (b n) -> p b n", b=B),
                     axis=mybir.AxisListType.X)
        e_ps = enps.tile([2, B], F32)
        TE.matmul(e_ps[:], small["i2e"][:], ech[:], start=True, stop=True)
        e_sb = entmp.tile([2, B], F32)
        V.tensor_copy(e_sb[:], e_ps[:])
        # exact fp8 split: e = ea + eb, ea = fp8(e), eb = e - ea (small int)
        V.tensor_copy(ea8[:], e_sb[:])
        ea32 = entmp.tile([2, B], F32)
        V.tensor_copy(ea32[:], ea8[:])
        eb = entmp.tile([2, B], F32)
        V.tensor_tensor(eb[:], e_sb[:], ea32[:], OP.subtract)
        V.tensor_copy(eb8[:], eb[:])
    for c in range(NC):
        V.tensor_copy(pay_sb[0:2, c, 512:516], ea8[:])
        V.tensor_copy(pay_sb[0:2, c, 516:520], eb8[:])

    # ================= attention: S = q (k^T v), spikes ==================
    with tc.tile_pool(name="nat", bufs=1) as nat, \
         tc.tile_pool(name="tps", bufs=2, space="PSUM") as tps, \
         tc.tile_pool(name="kvps", bufs=2, space="PSUM") as kvps, \
         tc.tile_pool(name="sps", bufs=2, space="PSUM") as sps, \
         tc.tile_pool(name="kvsb", bufs=1) as kvsb:
        knat = nat.tile([128, 32, 128], BF16)
        vnat = nat.tile([128, 32, 128], BF16)
        kvm = [kvsb.tile([128, 128], BF16, name=f"kvm{b}") for b in range(B)]

        def transposes(b):
            for half in range(2):
                tpk = tps.tile([128, 512], BF16, tag="tpk")
                tpv = tps.tile([128, 512], BF16, tag="tpv")
                for mi in range(4):
                    gt = b * 8 + half * 4 + mi
                    sl = slice(mi * 128, (mi + 1) * 128)
                    # one accumulation group per PSUM bank (disjoint
                    # columns): zero-on-first-write applies to the whole
                    # bank, then the remaining transposes land in it
                    TE.matmul(tpk[:, sl], spk[:, gt * 128:(gt + 1) * 128],
                              idn_sb[:], is_transpose=True,
                              start=(mi == 0), stop=(mi == 3),
                              skip_group_check=True)
                    TE.matmul(tpv[:, sl], spv[:, gt * 128:(gt + 1) * 128],
                              idn_sb[:], is_transpose=True,
                              start=(mi == 0), stop=(mi == 3),
                              skip_group_check=True)
                g0 = b * 8 + half * 4
                SC.activation(knat[:, g0:g0 + 4], tpk[:].rearrange(
                    "p (t m) -> p t m", t=4), AF.Copy)
                V.tensor_copy(vnat[:, g0:g0 + 4], tpv[:].rearrange(
                    "p (t m) -> p t m", t=4))

        def kv_stage(b):
            kv_ps = kvps.tile([128, 128], F32, tag="kv")
            for mt in range(8):
                TE.matmul(kv_ps[:], knat[:, b * 8 + mt], vnat[:, b * 8 + mt],
                          start=(mt == 0), stop=(mt == 7))
            V.tensor_tensor(kvm[b][:], kv_ps[:], small["mbd"][:], OP.mult)

        def s_stage(b):
            for nn in range(2):
                cdst = b * 2 + nn
                s_ps = sps.tile([128, 512], F32, tag="sps")
                TE.matmul(s_ps[:], kvm[b][:],
                          spq[:, cdst * 512:(cdst + 1) * 512],
                          start=True, stop=True)
                V.tensor_scalar(pay_sb[:, cdst, 0:512], s_ps[:], S_TH,
                                None, OP.is_ge)

        # software-pipelined over batches to keep PE busy
        transposes(0)
        transposes(1)
        kv_stage(0)
        transposes(2)
        kv_stage(1)
        s_stage(0)
        transposes(3)
        kv_stage(2)
        s_stage(1)
        kv_stage(3)
        s_stage(2)
        s_stage(3)

    # ================= AllToAll =================
    pay_d = dram.tile([NC, 128, PCOL], FP8)
    recv_d = dram.tile([NC, 128, PCOL], FP8)
    GP.dma_start(pay_d[:].rearrange("c p w -> p c w"), pay_sb[:])
    GP.collective_compute("AllToAll", OP.bypass,
                          ins=[pay_d.opt()], outs=[recv_d.opt()],
                          replica_groups=RG)

    # ================= gate + projection (own 512 tokens) ================
    with tc.tile_pool(name="phC", bufs=1) as phC:
        rsb = phC.tile([128, KP, 2, 512], FP8)
        rc_ap = recv_d[:].rearrange("(a r) p w -> p a r w", a=KP)
        SY.dma_start(rsb[:], rc_ap[:, :, :, 0:512])
        gx = phC.tile([128, KT], F32)
        with tc.tile_pool(name="gps", bufs=1, space="PSUM") as gps:
            # gather energies [8 s, 2 j, 8]: cols 0:4 = ea[b], 4:8 = eb[b];
            # head h = 2s + j
            er8 = phC.tile([NC, 2, 2 * B], FP8, name="er8")
            SC.dma_start(er8[:], recv_d[:, 0:2, 512:520])
            er = phC.tile([NC, 2, 2 * B], F32, name="er")
            V.tensor_copy(er[:], er8[:])
            e2 = phC.tile([NC, 2, B], F32, name="e2")
            V.tensor_tensor(e2[:], er[:, :, 0:B], er[:, :, B:2 * B], OP.add)
            V.tensor_tensor(e2[:], e2[:], small["bsel"][:].rearrange(
                "s (j b) -> s j b", j=2), OP.mult)
            emy = phC.tile([NC, 2], F32, name="emy")
            V.reduce_sum(emy[:], e2[:], axis=mybir.AxisListType.X)
            g_ps = gps.tile([H, 1], F32, name="g_ps")
            wgr_ap = small["wgr"][:].rearrange("s (j h) -> s j h", j=2)
            TE.matmul(g_ps[:], wgr_ap[:, 0], emy[:, 0:1],
                      start=True, stop=False)
            TE.matmul(g_ps[:], wgr_ap[:, 1], emy[:, 1:2],
                      start=False, stop=True)
            gate = phC.tile([H, 1], F32)
            V.tensor_scalar(gate[:], g_ps[:], small["bgr"][:], 0.5,
                            OP.add, OP.is_ge)
            rhs_t = phC.tile([H, KT], F32)
            V.tensor_scalar(rhs_t[:], small["i8t"][:], gate[:], None, OP.mult)
            gx_ps = gps.tile([128, KT], F32, name="gx_ps")
            TE.matmul(gx_ps[:], small["selp"][:], rhs_t[:],
                      start=True, stop=True)
            V.tensor_copy(gx[:], gx_ps[:])
        for i in range(KT):
            V.tensor_scalar(rsb[:, i // 2, i % 2], rsb[:, i // 2, i % 2],
                            gx[:, i:i + 1], None, OP.mult)

        osb = phC.tile([128, KT, 512], BF16)
        with tc.tile_pool(name="ppps", bufs=1, space="PSUM") as ppps:
            pp = [ppps.tile([128, 512], F32, name=f"pp{o}") for o in range(KT)]
            for o in range(KT):
                for a in range(KP):
                    TE.matmul(pp[o][:], wp_sb[:, a, :, o * 128:(o + 1) * 128],
                              rsb[:, a], start=(a == 0), stop=(a == KP - 1),
                              perf_mode=DR)
            # per-channel partial stats over own 512 tokens
            pst = phC.tile([128, KT, 6], F32)
            mvp = phC.tile([128, KT, 2], F32)
            for o in range(KT):
                V.bn_stats(pst[:, o], pp[o][:])
                V.bn_aggr(mvp[:, o], pst[:, o])
            arf = phC.tile([128, 2, KT], F32)
            V.tensor_scalar(arf[:, 0], mvp[:, :, 0], 512.0, None, OP.mult)
            V.tensor_tensor(arf[:, 1], mvp[:, :, 0], mvp[:, :, 0], OP.mult)
            V.tensor_tensor(arf[:, 1], arf[:, 1], mvp[:, :, 1], OP.add)
            V.tensor_scalar(arf[:, 1], arf[:, 1], 512.0, None, OP.mult)
            ps_d = dram.tile([128, 2 * KT], F32)
            pr_d = dram.tile([128, 2 * KT], F32)
            SY.dma_start(ps_d[:].rearrange("p (s t) -> p s t", s=2), arf[:])
            GP.collective_compute("AllReduce", OP.add,
                                  ins=[ps_d.opt()], outs=[pr_d.opt()],
                                  replica_groups=RG)
            arb = phC.tile([128, 2, KT], F32)
            SY.dma_start(arb[:], pr_d[:].rearrange("p (s t) -> p s t", s=2))
            mean = phC.tile([128, KT], F32)
            V.tensor_scalar(mean[:], arb[:, 0], 1.0 / TOK, None, OP.mult)
            var = phC.tile([128, KT], F32)
            V.tensor_tensor(var[:], mean[:], mean[:], OP.mult)
            ssm = phC.tile([128, KT], F32)
            V.tensor_scalar(ssm[:], arb[:, 1], 1.0 / TOK, None, OP.mult)
            V.tensor_tensor(var[:], ssm[:], var[:], OP.subtract)
            sdp = phC.tile([128, KT], F32)
            SC.activation(sdp[:], var[:], AF.Sqrt, bias=eps_sb[:])
            thrp = phC.tile([128, KT], F32)
            V.tensor_tensor(thrp[:], sdp[:], small["tp"][:], OP.mult)
            V.tensor_tensor(thrp[:], thrp[:], mean[:], OP.add)
            for o in range(KT):
                V.tensor_scalar(osb[:, o], pp[o][:], thrp[:, o:o + 1],
                                None, OP.is_ge)
        SY.dma_start(outT.ap().rearrange("p (t n) -> p t n", t=KT), osb[:])


def _prep_inputs(inputs):
    x = np.asarray(inputs["x"], np.float32)
    # xq[p, nc, kt, n] = x^T[kt*128+p, nc*512+n]
    xt = np.ascontiguousarray(x.reshape(TOK, D).T)
    xq = np.ascontiguousarray(
        xt.reshape(KT, 128, NC, 512).transpose(1, 2, 0, 3)
    ).reshape(128, NC * KT * 512).astype(F8)

    def wtile(W):
        # [p, kt*ncols + j] = W[j, kt*128+p]  (W already scaled/sliced)
        ncols = W.shape[0]
        return np.ascontiguousarray(
            W.T.reshape(KT, 128, ncols).transpose(1, 0, 2)
        ).reshape(128, KT * ncols)

    Wp = np.asarray(inputs["Wp"], np.float32)
    wp8 = wtile(8.0 * Wp).astype(F8)
    gp = np.asarray(inputs["gp"], np.float32)
    bp = np.asarray(inputs["betap"], np.float32)
    tp = np.ascontiguousarray(
        ((2.0 - bp) / gp).reshape(KT, 128).T).astype(np.float32)

    Wg = np.asarray(inputs["Wg"], np.float64)
    wgr0 = (Wg.reshape(H, HD, H).sum(axis=1).T / float(NSEQ)).astype(
        np.float32)                              # [h, h']
    # wgr[s, j*16+h'] = wgr0[2s+j, h']
    wgr = np.ascontiguousarray(
        wgr0.reshape(NC, 2, H).transpose(0, 1, 2).reshape(NC, 2 * H))
    bgr = np.asarray(inputs["bg"], np.float32).reshape(H, 1)

    i2e = np.zeros((CH, 2), np.float32)
    i2e[0:HD, 0] = 1.0
    i2e[HD:CH, 1] = 1.0
    selp = np.zeros((H, 128), np.float32)
    for h in range(H):
        selp[h, (h % 2) * HD:(h % 2 + 1) * HD] = 1.0
    i8t = np.zeros((H, KT), np.float32)
    for h in range(H):
        i8t[h, h // 2] = 1.0
    mbd = np.zeros((128, 128), np.float32)
    mbd[0:HD, 0:HD] = 1.0
    mbd[HD:128, HD:128] = 1.0
    idn = np.eye(128, dtype=BF)

    in_maps = []
    for c in range(NCORES):
        sl = slice(CH * c, CH * c + CH)
        bsel = np.zeros((NC, 2, B), np.float32)
        bsel[:, :, c // 2] = 1.0
        bsel = bsel.reshape(NC, 2 * B)
        m = {"xq": xq, "wp": wp8, "tp": tp, "wgr": wgr, "bgr": bgr,
             "i2e": i2e, "selp": selp, "i8t": i8t, "bsel": bsel,
             "mbd": mbd, "idn": idn}
        for nm in ("q", "k", "v"):
            W = np.asarray(inputs[f"W{nm}"], np.float32)
            m["w" + nm] = wtile(8.0 * W[sl, :]).astype(F8)
            g = np.asarray(inputs[f"g{nm}"], np.float32)[sl]
            be = np.asarray(inputs[f"beta{nm}"], np.float32)[sl]
            m["t" + nm] = ((2.0 - be) / g).reshape(CH, 1).astype(np.float32)
        in_maps.append(m)
    return in_maps


def _run(inputs, trace=False):
    if "nc" not in _CACHE:
        _CACHE["nc"] = _build()
    nc = _CACHE["nc"]
    in_maps = _prep_inputs(inputs)
    res = run_bass_kernel_spmd(nc, in_maps, core_ids=list(range(NCORES)),
                               trace=trace)
    out = np.empty((TOK, D), np.float32)
    for c in range(NCORES):
        blk = res.results[c]["outT"].reshape(128, KT, 512).astype(np.float32)
        out[c * 512:(c + 1) * 512, :] = blk.transpose(2, 1, 0).reshape(512, D)
    return out.reshape(B, NSEQ, D), res


def kernel(**inputs) -> np.ndarray:
    out, _ = _run(inputs, trace=False)
    return out


# revision 25
# speedup vs baseline: 2.6375x; 1.0631x over previous
"""Trainium2 Bass kernel for nn_LogicGatedSpikingSelfAttention (v2).

Structure (8 cores):
- q/k/v branches channel-sharded: core c owns output channels
  [128c, 128c+128) = heads 2c, 2c+1; BN stats fully local.
- Attention uses linearity (no softmax): S = q @ (k^T v) per (batch,
  head) - O(N*hd^2) instead of O(N^2*hd), integer-exact on {0,1}
  spikes (kv counts <= ~7 here, bf16-exact).
- Projection token-sharded: an AllToAll moves each core's [128ch x
  512tok] spike slices (~0.5MB) instead of an 8.4MB AllGather. Gate
  energies ride the payload as an exact fp8 hi+lo split; the gate is
  applied receiver-side to the proj rhs. Proj BN stats via an 8KB
  AllReduce of per-core (sum, sumsq).
- Branch + proj GEMMs run in fp8 e4m3 with DoubleRow perf mode
  (256-wide contraction per matmul). Weights pre-scaled by 8 on the
  host (spike threshold compare is scale-invariant; the linear bias
  cancels inside BatchNorm). Attention matmuls/transposes in bf16.
"""
import numpy as np
import ml_dtypes

import concourse.bass as bass
import concourse.bacc as bacc
import concourse.tile as tile
from concourse import mybir
from concourse.bass_utils import run_bass_kernel_spmd

NCORES = 8
B, NSEQ, D, H = 4, 1024, 1024, 16
HD = D // H            # 64 head dim
CH = D // NCORES       # 128 channels per core
TOK = B * NSEQ         # 4096 tokens
KT = D // 128          # 8 contraction tiles of 128
KP = KT // 2           # 4 DoubleRow contraction pairs of 256
NC = TOK // 512        # 8 token chunks of 512
EPS = 1e-5
S_TH = float(2.0 ** 0.75)   # x_attn spike <=> S >= hd**0.125 = 2^0.75
PCOL = 520                  # payload row: 512 spikes + 8 energy cols
F32 = mybir.dt.float32
BF16 = mybir.dt.bfloat16
FP8 = mybir.dt.float8e4
BF = ml_dtypes.bfloat16
F8 = mybir.dt.np(mybir.dt.float8e4)
DR = mybir.MatmulPerfMode.DoubleRow

_CACHE = {}


def _build():
    nc = bacc.Bacc("TRN2", target_bir_lowering=False, debug=False,
                   num_devices=NCORES)
    inp = {}
    def din(name, shape, dt):
        inp[name] = nc.dram_tensor(name, shape, dt, kind="ExternalInput")

    din("xq", [128, NC * KT * 512], FP8)     # [p, (nc kt n)] = xT pre-tiled
    din("wq", [128, KT * CH], FP8)           # [p, (kt j)] = 8*W[c0+j, kt*128+p]
    din("wk", [128, KT * CH], FP8)
    din("wv", [128, KT * CH], FP8)
    din("wp", [128, KT * D], FP8)            # full Wp^T: [p,(ct j)] = 8*Wp[j, ct*128+p]
    for nm in ("tq", "tk", "tv"):
        din(nm, [CH, 1], F32)                # (2-beta)/gamma, core's channel slice
    din("tp", [CH, KT], F32)                 # proj (2-beta)/gamma, all 1024 ch
    din("wgr", [NC, 2 * H], F32)             # [s, j*16+h'] = wgr0[2s+j, h']
    din("bgr", [H, 1], F32)
    din("i2e", [CH, 2], F32)                 # [p, j] = (p//64 == j)
    din("selp", [H, 128], F32)               # [h, p] = (p//64 == h%2)
    din("i8t", [H, KT], F32)                 # [h, t] = (h//2 == t)
    din("bsel", [NC, 2 * B], F32)            # per-core one-hot of own batch x2
    din("mbd", [128, 128], F32)              # block-diag 64x64 ones mask
    din("idn", [128, 128], BF16)             # identity for PE transpose
    outT = nc.dram_tensor("outT", [128, KT * 512], BF16, kind="ExternalOutput")

    with tile.TileContext(nc) as tc:
        with tc.tile_pool(name="consts", bufs=1) as consts, \
             tc.tile_pool(name="spk", bufs=1) as spk_pool, \
             tc.tile_pool(name="dram", bufs=1, space="DRAM") as dram:
            _body(tc, inp, outT, consts, spk_pool, dram)
    nc.compile()
    return nc


def _body(tc, inp, outT, consts, spk_pool, dram):
    nc = tc.nc
    V, SC, GP, TE, SY = nc.vector, nc.scalar, nc.gpsimd, nc.tensor, nc.sync
    AF = mybir.ActivationFunctionType
    OP = mybir.AluOpType
    RG = [list(range(NCORES))]

    # ---- constants / weights to SBUF ----
    w_sb = {}
    for i, nm in enumerate(("wq", "wk", "wv")):
        t = consts.tile([128, KP, 2, CH], FP8, name=f"{nm}_sb")
        SC.dma_start(t[:], inp[nm].ap().rearrange(
            "p (a r m) -> p a r m", a=KP, r=2))
        w_sb[nm] = t
    wp_sb = consts.tile([128, KP, 2, D], FP8)
    small = {}
    for nm in ("tq", "tk", "tv", "tp", "wgr", "bgr", "i2e", "selp", "i8t",
               "bsel", "mbd"):
        t = consts.tile(list(inp[nm].shape), F32, name=f"{nm}_sb")
        GP.dma_start(t[:], inp[nm].ap())
        small[nm] = t
    idn_sb = consts.tile([128, 128], BF16)
    GP.dma_start(idn_sb[:], inp["idn"].ap())
    eps_sb = consts.tile([128, 1], F32)
    V.memset(eps_sb[:], EPS)

    # ---- persistent spike / payload tensors ----
    spq = spk_pool.tile([128, TOK], BF16, name="spq")
    spk = spk_pool.tile([128, TOK], BF16, name="spk")
    spv = spk_pool.tile([128, TOK], BF16, name="spv")
    pay_sb = spk_pool.tile([128, NC, PCOL], FP8, name="pay_sb")
    V.memset(pay_sb[:, :, 512:PCOL], 0.0)

    # ================= branches (k, v, q) =================
    brs = ("k", "v", "q")
    with tc.tile_pool(name="xqp", bufs=1) as xqp, \
         tc.tile_pool(name="ybig", bufs=1) as ybig, \
         tc.tile_pool(name="stp", bufs=1) as stp:
        xq = xqp.tile([128, NC, KP, 2, 512], FP8)
        xq_ap = inp["xq"].ap().rearrange(
            "p (c a r n) -> p c a r n", c=NC, a=KP, r=2)
        for c in range(NC):
            (SY, SC)[c % 2].dma_start(xq[:, c], xq_ap[:, c])
        SC.dma_start(wp_sb[:], inp["wp"].ap().rearrange(
            "p (a r m) -> p a r m", a=KP, r=2))

        Y = {nm: ybig.tile([128, TOK], BF16, name=f"Y{nm}") for nm in brs}
        st = {nm: stp.tile([128, NC, 6], F32, name=f"st{nm}") for nm in brs}
        with tc.tile_pool(name="brps", bufs=1, space="PSUM") as brps:
            ps = [brps.tile([128, 512], F32, name=f"ps{i}") for i in range(6)]
            for c in range(NC):
                g = c % 2
                for bi, nm in enumerate(brs):
                    p = ps[g * 3 + bi]
                    for a in range(KP):
                        TE.matmul(p[:], w_sb["w" + nm][:, a], xq[:, c, a],
                                  start=(a == 0), stop=(a == KP - 1),
                                  perf_mode=DR)
                    SC.activation(Y[nm][:, c * 512:(c + 1) * 512], p[:],
                                  AF.Copy)
                    V.bn_stats(st[nm][:, c], p[:])

        # stats -> per-channel thresholds -> spikes
        sp = {"k": spk, "v": spv, "q": spq}
        for nm in brs:
            mv = stp.tile([128, 2], F32, name=f"mv{nm}")
            V.bn_aggr(mv[:], st[nm][:])
            sd = stp.tile([128, 1], F32, name=f"sd{nm}")
            SC.activation(sd[:], mv[:, 1:2], AF.Sqrt, bias=eps_sb[:])
            th = stp.tile([128, 1], F32, name=f"th{nm}")
            V.tensor_tensor(th[:], sd[:], small["t" + nm][:], OP.mult)
            V.tensor_tensor(th[:], th[:], mv[:, 0:1], OP.add)
            V.tensor_scalar(sp[nm][:], Y[nm][:], th[:], None, OP.is_ge)

    # ================= attention: S = q (k^T v), spikes ==================
    with tc.tile_pool(name="nat", bufs=1) as nat, \
         tc.tile_pool(name="tps", bufs=2, space="PSUM") as tps, \
         tc.tile_pool(name="kvps", bufs=2, space="PSUM") as kvps, \
         tc.tile_pool(name="sps", bufs=2, space="PSUM") as sps, \
         tc.tile_pool(name="kvsb", bufs=1) as kvsb:
        knat = nat.tile([128, 32, 128], BF16)
        vnat = nat.tile([128, 32, 128], BF16)
        kvm = [kvsb.tile([128, 128], BF16, name=f"kvm{b}") for b in range(B)]

        def transposes(b):
            for half in range(2):
                tpk = tps.tile([128, 512], BF16, tag="tpk")
                tpv = tps.tile([128, 512], BF16, tag="tpv")
                for mi in range(4):
                    gt = b * 8 + half * 4 + mi
                    sl = slice(mi * 128, (mi + 1) * 128)
                    # one accumulation group per PSUM bank (disjoint
                    # columns): zero-on-first-write applies to the whole
                    # bank, then the remaining transposes land in it
                    TE.matmul(tpk[:, sl], spk[:, gt * 128:(gt + 1) * 128],
                              idn_sb[:], is_transpose=True,
                              start=(mi == 0), stop=(mi == 3),
                              skip_group_check=True)
                    TE.matmul(tpv[:, sl], spv[:, gt * 128:(gt + 1) * 128],
                              idn_sb[:], is_transpose=True,
                              start=(mi == 0), stop=(mi == 3),
                              skip_group_check=True)
                g0 = b * 8 + half * 4
                SC.activation(knat[:, g0:g0 + 4], tpk[:].rearrange(
                    "p (t m) -> p t m", t=4), AF.Copy)
                V.tensor_copy(vnat[:, g0:g0 + 4], tpv[:].rearrange(
                    "p (t m) -> p t m", t=4))

        def kv_stage(b):
            kv_ps = kvps.tile([128, 128], F32, tag="kv")
            for mt in range(8):
                TE.matmul(kv_ps[:], knat[:, b * 8 + mt], vnat[:, b * 8 + mt],
                          start=(mt == 0), stop=(mt == 7))
            V.tensor_tensor(kvm[b][:], kv_ps[:], small["mbd"][:], OP.mult)

        def s_stage(b):
            for nn in range(2):
                cdst = b * 2 + nn
                s_ps = sps.tile([128, 512], F32, tag="sps")
                TE.matmul(s_ps[:], kvm[b][:],
                          spq[:, cdst * 512:(cdst + 1) * 512],
                          start=True, stop=True)
                V.tensor_scalar(pay_sb[:, cdst, 0:512], s_ps[:], S_TH,
                                None, OP.is_ge)

        # software-pipelined over batches to keep PE busy
        transposes(0)
        transposes(1)
        kv_stage(0)
        transposes(2)
        kv_stage(1)
        s_stage(0)
        transposes(3)
        kv_stage(2)
        s_stage(1)
        kv_stage(3)
        s_stage(2)
        s_stage(3)

    # ================= energy (own 2 heads, all batches) =================
    # placed after attention so the PE queue is not blocked in-order on
    # the vector-engine energy reduction
    ea8 = spk_pool.tile([2, B], FP8, name="ea8")
    eb8 = spk_pool.tile([2, B], FP8, name="eb8")
    with tc.tile_pool(name="entmp", bufs=1) as entmp, \
         tc.tile_pool(name="enps", bufs=1, space="PSUM") as enps:
        prod = entmp.tile([128, TOK], BF16)
        V.tensor_tensor(prod[:], spq[:], spk[:], OP.mult)
        ech = entmp.tile([128, B], F32)
        V.reduce_sum(ech[:], prod[:].rearrange("p (b n) -> p b n", b=B),
                     axis=mybir.AxisListType.X)
        e_ps = enps.tile([2, B], F32)
        TE.matmul(e_ps[:], small["i2e"][:], ech[:], start=True, stop=True)
        e_sb = entmp.tile([2, B], F32)
        V.tensor_copy(e_sb[:], e_ps[:])
        # exact fp8 split: e = ea + eb, ea = fp8(e), eb = e - ea (small int)
        V.tensor_copy(ea8[:], e_sb[:])
        ea32 = entmp.tile([2, B], F32)
        V.tensor_copy(ea32[:], ea8[:])
        eb = entmp.tile([2, B], F32)
        V.tensor_tensor(eb[:], e_sb[:], ea32[:], OP.subtract)
        V.tensor_copy(eb8[:], eb[:])
    for c in range(NC):
        V.tensor_copy(pay_sb[0:2, c, 512:516], ea8[:])
        V.tensor_copy(pay_sb[0:2, c, 516:520], eb8[:])

    # ================= AllToAll =================
    pay_d = dram.tile([NC, 128, PCOL], FP8)
    recv_d = dram.tile([NC, 128, PCOL], FP8)
    GP.dma_start(pay_d[:].rearrange("c p w -> p c w"), pay_sb[:])
    GP.collective_compute("AllToAll", OP.bypass,
                          ins=[pay_d.opt()], outs=[recv_d.opt()],
                          replica_groups=RG)

    # ================= gate + projection (own 512 tokens) ================
    with tc.tile_pool(name="phC", bufs=1) as phC:
        rsb = phC.tile([128, KP, 2, 512], FP8)
        rc_ap = recv_d[:].rearrange("(a r) p w -> p a r w", a=KP)
        SY.dma_start(rsb[:], rc_ap[:, :, :, 0:512])
        gx = phC.tile([128, KT], F32)
        with tc.tile_pool(name="gps", bufs=1, space="PSUM") as gps:
            # gather energies [8 s, 2 j, 8]: cols 0:4 = ea[b], 4:8 = eb[b];
            # head h = 2s + j
            er8 = phC.tile([NC, 2, 2 * B], FP8, name="er8")
            SC.dma_start(er8[:], recv_d[:, 0:2, 512:520])
            er = phC.tile([NC, 2, 2 * B], F32, name="er")
            V.tensor_copy(er[:], er8[:])
            e2 = phC.tile([NC, 2, B], F32, name="e2")
            V.tensor_tensor(e2[:], er[:, :, 0:B], er[:, :, B:2 * B], OP.add)
            V.tensor_tensor(e2[:], e2[:], small["bsel"][:].rearrange(
                "s (j b) -> s j b", j=2), OP.mult)
            emy = phC.tile([NC, 2], F32, name="emy")
            V.reduce_sum(emy[:], e2[:], axis=mybir.AxisListType.X)
            g_ps = gps.tile([H, 1], F32, name="g_ps")
            wgr_ap = small["wgr"][:].rearrange("s (j h) -> s j h", j=2)
            TE.matmul(g_ps[:], wgr_ap[:, 0], emy[:, 0:1],
                      start=True, stop=False)
            TE.matmul(g_ps[:], wgr_ap[:, 1], emy[:, 1:2],
                      start=False, stop=True)
            gate = phC.tile([H, 1], F32)
            V.tensor_scalar(gate[:], g_ps[:], small["bgr"][:], 0.5,
                            OP.add, OP.is_ge)
            rhs_t = phC.tile([H, KT], F32)
            V.tensor_scalar(rhs_t[:], small["i8t"][:], gate[:], None, OP.mult)
            gx_ps = gps.tile([128, KT], F32, name="gx_ps")
            TE.matmul(gx_ps[:], small["selp"][:], rhs_t[:],
                      start=True, stop=True)
            V.tensor_copy(gx[:], gx_ps[:])
        for i in range(KT):
            V.tensor_scalar(rsb[:, i // 2, i % 2], rsb[:, i // 2, i % 2],
                            gx[:, i:i + 1], None, OP.mult)

        osb = phC.tile([128, KT, 512], BF16)
        with tc.tile_pool(name="ppps", bufs=1, space="PSUM") as ppps:
            pp = [ppps.tile([128, 512], F32, name=f"pp{o}") for o in range(KT)]
            for o in range(KT):
                for a in range(KP):
                    TE.matmul(pp[o][:], wp_sb[:, a, :, o * 128:(o + 1) * 128],
                              rsb[:, a], start=(a == 0), stop=(a == KP - 1),
                              perf_mode=DR)
            # per-channel partial stats over own 512 tokens
            pst = phC.tile([128, KT, 6], F32)
            mvp = phC.tile([128, KT, 2], F32)
            for o in range(KT):
                V.bn_stats(pst[:, o], pp[o][:])
                V.bn_aggr(mvp[:, o], pst[:, o])
            arf = phC.tile([128, 2, KT], F32)
            V.tensor_scalar(arf[:, 0], mvp[:, :, 0], 512.0, None, OP.mult)
            V.tensor_tensor(arf[:, 1], mvp[:, :, 0], mvp[:, :, 0], OP.mult)
            V.tensor_tensor(arf[:, 1], arf[:, 1], mvp[:, :, 1], OP.add)
            V.tensor_scalar(arf[:, 1], arf[:, 1], 512.0, None, OP.mult)
            ps_d = dram.tile([128, 2 * KT], F32)
            pr_d = dram.tile([128, 2 * KT], F32)
            SY.dma_start(ps_d[:].rearrange("p (s t) -> p s t", s=2), arf[:])
            GP.collective_compute("AllReduce", OP.add,
                                  ins=[ps_d.opt()], outs=[pr_d.opt()],
                                  replica_groups=RG)
            arb = phC.tile([128, 2, KT], F32)
            SY.dma_start(arb[:], pr_d[:].rearrange("p (s t) -> p s t", s=2))
            mean = phC.tile([128, KT], F32)
            V.tensor_scalar(mean[:], arb[:, 0], 1.0 / TOK, None, OP.mult)
            var = phC.tile([128, KT], F32)
            V.tensor_tensor(var[:], mean[:], mean[:], OP.mult)
            ssm = phC.tile([128, KT], F32)
            V.tensor_scalar(ssm[:], arb[:, 1], 1.0 / TOK, None, OP.mult)
            V.tensor_tensor(var[:], ssm[:], var[:], OP.subtract)
            sdp = phC.tile([128, KT], F32)
            SC.activation(sdp[:], var[:], AF.Sqrt, bias=eps_sb[:])
            thrp = phC.tile([128, KT], F32)
            V.tensor_tensor(thrp[:], sdp[:], small["tp"][:], OP.mult)
            V.tensor_tensor(thrp[:], thrp[:], mean[:], OP.add)
            nthr = phC.tile([128, KT], F32)
            V.tensor_scalar(nthr[:], thrp[:], -1.0, None, OP.mult)
            for o in range(KT):
                if o % 2 == 0:
                    V.tensor_scalar(osb[:, o], pp[o][:], thrp[:, o:o + 1],
                                    None, OP.is_ge)
                else:
                    # sign(pp - thr) in {-1,0,1}; host maps > 0 -> spike
                    SC.activation(osb[:, o], pp[o][:], AF.Sign,
                                  bias=nthr[:, o:o + 1])
        SY.dma_start(outT.ap().rearrange("p (t n) -> p t n", t=KT), osb[:])


def _prep_inputs(inputs):
    x = np.asarray(inputs["x"], np.float32)
    # xq[p, nc, kt, n] = x^T[kt*128+p, nc*512+n]
    xt = np.ascontiguousarray(x.reshape(TOK, D).T)
    xq = np.ascontiguousarray(
        xt.reshape(KT, 128, NC, 512).transpose(1, 2, 0, 3)
    ).reshape(128, NC * KT * 512).astype(F8)

    def wtile(W):
        # [p, kt*ncols + j] = W[j, kt*128+p]  (W already scaled/sliced)
        ncols = W.shape[0]
        return np.ascontiguousarray(
            W.T.reshape(KT, 128, ncols).transpose(1, 0, 2)
        ).reshape(128, KT * ncols)

    Wp = np.asarray(inputs["Wp"], np.float32)
    wp8 = wtile(8.0 * Wp).astype(F8)
    gp = np.asarray(inputs["gp"], np.float32)
    bp = np.asarray(inputs["betap"], np.float32)
    tp = np.ascontiguousarray(
        ((2.0 - bp) / gp).reshape(KT, 128).T).astype(np.float32)

    Wg = np.asarray(inputs["Wg"], np.float64)
    wgr0 = (Wg.reshape(H, HD, H).sum(axis=1).T / float(NSEQ)).astype(
        np.float32)                              # [h, h']
    # wgr[s, j*16+h'] = wgr0[2s+j, h']
    wgr = np.ascontiguousarray(
        wgr0.reshape(NC, 2, H).transpose(0, 1, 2).reshape(NC, 2 * H))
    bgr = np.asarray(inputs["bg"], np.float32).reshape(H, 1)

    i2e = np.zeros((CH, 2), np.float32)
    i2e[0:HD, 0] = 1.0
    i2e[HD:CH, 1] = 1.0
    selp = np.zeros((H, 128), np.float32)
    for h in range(H):
        selp[h, (h % 2) * HD:(h % 2 + 1) * HD] = 1.0
    i8t = np.zeros((H, KT), np.float32)
    for h in range(H):
        i8t[h, h // 2] = 1.0
    mbd = np.zeros((128, 128), np.float32)
    mbd[0:HD, 0:HD] = 1.0
    mbd[HD:128, HD:128] = 1.0
    idn = np.eye(128, dtype=BF)

    in_maps = []
    for c in range(NCORES):
        sl = slice(CH * c, CH * c + CH)
        bsel = np.zeros((NC, 2, B), np.float32)
        bsel[:, :, c // 2] = 1.0
        bsel = bsel.reshape(NC, 2 * B)
        m = {"xq": xq, "wp": wp8, "tp": tp, "wgr": wgr, "bgr": bgr,
             "i2e": i2e, "selp": selp, "i8t": i8t, "bsel": bsel,
             "mbd": mbd, "idn": idn}
        for nm in ("q", "k", "v"):
            W = np.asarray(inputs[f"W{nm}"], np.float32)
            m["w" + nm] = wtile(8.0 * W[sl, :]).astype(F8)
            g = np.asarray(inputs[f"g{nm}"], np.float32)[sl]
            be = np.asarray(inputs[f"beta{nm}"], np.float32)[sl]
            m["t" + nm] = ((2.0 - be) / g).reshape(CH, 1).astype(np.float32)
        in_maps.append(m)
    return in_maps


def _run(inputs, trace=False):
    if "nc" not in _CACHE:
        _CACHE["nc"] = _build()
    nc = _CACHE["nc"]
    in_maps = _prep_inputs(inputs)
    res = run_bass_kernel_spmd(nc, in_maps, core_ids=list(range(NCORES)),
                               trace=trace)
    out = np.empty((TOK, D), np.float32)
    for c in range(NCORES):
        blk = res.results[c]["outT"].reshape(128, KT, 512).astype(np.float32)
        # odd out-tiles hold sign(Yp - thr) from the scalar engine
        blk[:, 1::2, :] = (blk[:, 1::2, :] > 0).astype(np.float32)
        out[c * 512:(c + 1) * 512, :] = blk.transpose(2, 1, 0).reshape(512, D)
    return out.reshape(B, NSEQ, D), res


def kernel(**inputs) -> np.ndarray:
    out, _ = _run(inputs, trace=False)
    return out
